# revision 1
# baseline (speedup 1.0000x reference)
"""GCN message-passing kernel for Trainium2 (8 NeuronCores, SPMD).

Math (reference):
    h    = gcn_conv(x, edge_index, W1, b1)   # sym-normalized scatter-add, self-loops
    h    = elu(h)
    pool = segment_sum(h, batch)             # 1024 graphs
    out  = pool @ W2 + b2                    # [1024, 1]

Key algebraic restructure: W1 is applied AFTER aggregation (linearity):
    z_i  = dis_i * ( sum_{j->i} dis_j * x_j + dis_i * x_i )      # 4-dim aggregation
    h_i  = z_i @ W1 + b1
so the gather/scatter payload is 4 floats (8 bytes in bf16), not 64.

Distribution: shard by graph (128 graphs/core); batch is sorted so node ranges
are contiguous per core.  Edges are assigned to the core owning their TARGET
node.  Each core builds the full s-table (s = dis*x, bf16) privately, then
gathers s[row] via indirect DMA in target-sorted order and segment-sums via
one-hot matmuls into 128-node windows.

Host does integer index preprocessing only (sort, bincount, layout); all
floating-point math runs on-device.
"""

import os
import sys

sys.path.insert(0, "/opt/trn_rl_repo")

import numpy as np
import ml_dtypes

import concourse.bass as bass
import concourse.bacc as bacc
import concourse.mybir as mybir
import concourse.tile as tile
from concourse.bass import IndirectOffsetOnAxis

F32 = mybir.dt.float32
BF16 = mybir.dt.bfloat16
I32 = mybir.dt.int32

NCORES = 8
LAST_RESULTS = None
F = 4          # input features
H = 64         # hidden
W = 128        # nodes per window
WG = 4         # windows per group


# --------------------------------------------------------------------------
# Host-side index preprocessing (integers only)
# --------------------------------------------------------------------------

def host_prep(x, edge_index, batch, n_graphs):
    """Compute the per-core index layout.  Returns cfg dict + per-core arrays."""
    N = x.shape[0]
    N_pad = ((N + 127) // 128) * 128
    E = edge_index.shape[1]
    assert n_graphs % NCORES == 0
    gpc = n_graphs // NCORES  # graphs per core (= 128 for the real problem)
    assert gpc <= 128

    row = edge_index[0].astype(np.int64)
    col = edge_index[1].astype(np.int64)
    batch = batch.astype(np.int64)

    deg = np.bincount(col, minlength=N).astype(np.int32) + 1  # + self loop

    # real edges only; self-loops are handled by a dedicated regular-DMA path
    rowA = row
    colA = col

    # core ownership by graph; node ranges per core (batch sorted)
    gb = np.searchsorted(batch, np.arange(0, n_graphs + 1, gpc))  # [NCORES+1]
    node_start, node_end = gb[:-1], gb[1:]
    nodes_per_core = node_end - node_start

    # windows per core (padded to WG multiple), shared across cores
    NW = int(np.ceil(nodes_per_core.max() / W))
    NW = ((NW + WG - 1) // WG) * WG

    # sort all edges by target node (stable not needed)
    order = np.argsort(colA, kind="stable")
    colS = colA[order]
    rowS = rowA[order]

    # per-edge core and local window / in-window rank
    core_of = np.searchsorted(node_start, colS, side="right") - 1
    ln = colS - node_start[core_of]           # local node index on its core
    wloc = ln // W                            # local window
    # rank of edge within its (core, window):
    gw = core_of * NW + wloc                  # global window id (sorted ascending)
    # edges are sorted by colS so gw is non-decreasing
    wstart = np.searchsorted(gw, np.arange(NCORES * NW))
    wcount = np.diff(np.concatenate([wstart, [len(gw)]]))
    rank = np.arange(len(gw)) - wstart[gw]

    C = int(np.ceil(wcount.max() / 128))      # chunks per window (uniform)
    B = C * WG                                # chunks per group
    NG = NW // WG
    NCHUNK = NW * C

    # slot position of each edge in the padded [NCORES, NCHUNK, 128] layout
    chunk_in_w = rank // 128
    slot = rank % 128
    chunk_id = wloc * C + chunk_in_w          # within core

    TABLE_ROWS = N_pad + 128                  # zero rows at the end
    gidx = np.full((NCORES, NCHUNK, 128), N_pad, dtype=np.int32)  # pad -> zero row
    lcol = np.zeros((NCORES, NCHUNK, 128), dtype=np.int32)
    gidx[core_of, chunk_id, slot] = rowS
    lcol[core_of, chunk_id, slot] = ln % W

    # [NCORES, NG, 128(slot), B] transposed layouts for direct [128, B] DMA
    def regroup(a):
        a = a.reshape(NCORES, NG, B, 128)
        return np.ascontiguousarray(a.transpose(0, 1, 3, 2))

    gidx_t = regroup(gidx)
    lcol_t = regroup(lcol).astype(np.float32)

    # per-node local graph id, padded windows -> -1
    lbat = np.full((NCORES, NW * W), -1.0, dtype=np.float64)
    deg4 = np.ones((NCORES, NW * W), dtype=np.int32)
    for c in range(NCORES):
        n0, n1 = node_start[c], node_end[c]
        nn = n1 - n0
        lbat[c, :nn] = batch[n0:n1] - c * gpc
        deg4[c, :nn] = deg[n0:n1]
    lbat_t = np.ascontiguousarray(
        lbat.reshape(NCORES, NG, WG, 128).transpose(0, 1, 3, 2)
    ).astype(np.float32)
    deg4r = np.ascontiguousarray(
        np.broadcast_to(deg4[:, None, :], (NCORES, F, NW * W))
    )

    iota = np.broadcast_to(np.arange(128, dtype=np.float64), (128, 128))
    iota = np.ascontiguousarray(iota).astype(np.float32)

    # per-core own-node features (node-major, padded) for the self-loop path
    x_own = np.zeros((NCORES, NW * W, x.shape[1]), np.float32)
    for c in range(NCORES):
        n0, n1 = node_start[c], node_end[c]
        x_own[c, :n1 - n0] = x[n0:n1]

    ident = np.eye(128, dtype=np.float64).astype(ml_dtypes.bfloat16)

    x_pad = x
    deg_pad = deg
    if N_pad != N:
        x_pad = np.concatenate(
            [x, np.zeros((N_pad - N, x.shape[1]), np.float32)])
        deg_pad = np.concatenate(
            [deg, np.ones(N_pad - N, np.int32)])

    cfg = dict(N=N, N_pad=N_pad, E=E, NW=NW, NG=NG, C=C, B=B, NCHUNK=NCHUNK,
               TABLE_ROWS=TABLE_ROWS, gpc=gpc)
    percore = dict(gidx_t=gidx_t, lcol_t=lcol_t, lbat_t=lbat_t, deg4=deg4r,
                   x_own=x_own)
    shared = dict(deg_full=deg_pad, iota=iota, x_pad=x_pad, ident=ident)
    return cfg, percore, shared


# --------------------------------------------------------------------------
# Device kernel builder
# --------------------------------------------------------------------------

def build_kernel(nc, cfg):
    N, NW, NG, C, B = cfg["N_pad"], cfg["NW"], cfg["NG"], cfg["C"], cfg["B"]
    TABLE_ROWS = cfg["TABLE_ROWS"]

    x_full = nc.declare_dram_parameter("x_full", [N, F], F32, isOutput=False)
    deg_full = nc.declare_dram_parameter("deg_full", [N], I32, isOutput=False)
    deg4 = nc.declare_dram_parameter("deg4", [F, NW * W], I32, isOutput=False)
    gidx_t = nc.declare_dram_parameter("gidx_t", [NG, 128, B], I32, isOutput=False)
    lcol_t = nc.declare_dram_parameter("lcol_t", [NG, 128, B], F32, isOutput=False)
    lbat_t = nc.declare_dram_parameter("lbat_t", [NG, 128, WG], F32, isOutput=False)
    iota_in = nc.declare_dram_parameter("iota", [128, 128], F32, isOutput=False)
    ident_in = nc.declare_dram_parameter("ident", [128, 128], BF16, isOutput=False)
    x_own_in = nc.declare_dram_parameter("x_own", [NW * W, F], F32, isOutput=False)
    W1_in = nc.declare_dram_parameter("W1", [F, H], F32, isOutput=False)
    b1_in = nc.declare_dram_parameter("b1", [H, 1], F32, isOutput=False)
    W2_in = nc.declare_dram_parameter("W2", [H, 1], F32, isOutput=False)
    b2_in = nc.declare_dram_parameter("b2", [1, 1], F32, isOutput=False)
    outp = nc.declare_dram_parameter("outp", [1, 128], F32, isOutput=True)
    dbg = cfg.get("debug", False)
    if dbg:
        NWW = NW * W
        dbg_m = nc.declare_dram_parameter("dbg_m", [128, cfg["B"] * F], F32, isOutput=True)
        dbg_oh = nc.declare_dram_parameter("dbg_oh", [128, cfg["B"] * W], F32, isOutput=True)
        dbg_bh = nc.declare_dram_parameter("dbg_bh", [128, WG * 128], F32, isOutput=True)
        dbg_zp = nc.declare_dram_parameter("dbg_zp", [F, WG * W], F32, isOutput=True)
        dbg_zd = nc.declare_dram_parameter("dbg_zd", [F, WG * W], F32, isOutput=True)
        dbg_el = nc.declare_dram_parameter("dbg_el", [H, WG * W], F32, isOutput=True)
        dbg_qs = nc.declare_dram_parameter("dbg_qs", [128, WG], F32, isOutput=True)

    s_dram = nc.dram_tensor("s_table", [TABLE_ROWS, F], BF16)
    s_own_dram = nc.dram_tensor("s_own", [NW * W, F], BF16)

    # s-build tiling: [128, DEGW] node chunks; DEGW divides N/128
    NPP = N // 128                    # nodes per partition across all chunks
    DEGW = max(d for d in range(1, min(NPP, 2048) + 1) if NPP % d == 0)
    SCH = NPP // DEGW                 # number of s-build chunks
    SBW = DEGW * F

    with tile.TileContext(nc) as tc:
        with (
            tc.tile_pool(name="consts", bufs=1) as cpool,
            tc.tile_pool(name="sbuild", bufs=2) as spool,
            tc.tile_pool(name="main", bufs=3) as mpool,
            tc.tile_pool(name="psum_w", bufs=2, space="PSUM") as pw,
            tc.tile_pool(name="psum_acc", bufs=1, space="PSUM") as pacc,
        ):
            # ---- constants ----
            iota_sb = cpool.tile([128, 128], F32)
            nc.sync.dma_start(out=iota_sb[:], in_=iota_in[:])
            ident_sb = cpool.tile([128, 128], BF16)
            nc.sync.dma_start(out=ident_sb[:], in_=ident_in[:])
            w1f = cpool.tile([F, H], F32)
            nc.sync.dma_start(out=w1f[:], in_=W1_in[:])
            w1b = cpool.tile([F, H], BF16)
            nc.vector.tensor_copy(out=w1b[:], in_=w1f[:])
            w2f = cpool.tile([H, 1], F32)
            nc.sync.dma_start(out=w2f[:], in_=W2_in[:])
            w2b = cpool.tile([H, 1], BF16)
            nc.vector.tensor_copy(out=w2b[:], in_=w2f[:])
            b1dup = cpool.tile([H, 1], F32)
            nc.sync.dma_start(out=b1dup[:], in_=b1_in[:])
            b2sb = cpool.tile([1, 1], F32)
            nc.sync.dma_start(out=b2sb[:], in_=b2_in[:])

            # ---- phase 1: build s table (s = rsqrt(deg) * x, bf16) ----
            xv = x_full[:].rearrange("n f -> (n f)").rearrange("(a b) -> a b", b=SBW)
            dv = deg_full[:].rearrange("(a b) -> a b", b=DEGW)
            sv = s_dram[0:N, :].rearrange("n f -> (n f)").rearrange("(a b) -> a b", b=SBW)
            for k in range(SCH):
                xt = spool.tile([128, DEGW, F], F32, tag="xt")
                nc.sync.dma_start(
                    out=xt[:].rearrange("p a b -> p (a b)"),
                    in_=xv[k * 128:(k + 1) * 128, :])
                dti = spool.tile([128, DEGW], I32, tag="dti")
                nc.sync.dma_start(out=dti[:], in_=dv[k * 128:(k + 1) * 128, :])
                dtf = spool.tile([128, DEGW], F32, tag="dtf")
                nc.vector.tensor_copy(out=dtf[:], in_=dti[:])
                rec = spool.tile([128, DEGW], F32, tag="rec")
                nc.vector.reciprocal(out=rec[:], in_=dtf[:])
                dis = spool.tile([128, DEGW], F32, tag="dis")
                nc.scalar.activation(dis[:], rec[:],
                                     mybir.ActivationFunctionType.Sqrt)
                st = spool.tile([128, DEGW, F], BF16, tag="st")
                nc.vector.tensor_mul(
                    out=st[:],
                    in0=xt[:],
                    in1=dis[:].unsqueeze(2).to_broadcast([128, DEGW, F]))
                nc.sync.dma_start(
                    out=sv[k * 128:(k + 1) * 128, :],
                    in_=st[:].rearrange("p a b -> p (a b)"))
            zt = spool.tile([128, F], BF16, tag="zt")
            nc.vector.memset(zt[:], 0)
            nc.sync.dma_start(out=s_dram[N:N + 128, :], in_=zt[:])

            # own-stripe s in core-local layout (for the self-loop path)
            NWW = NW * W
            OD = NWW // 128                   # own nodes per partition
            xov = x_own_in[:].rearrange("n f -> (n f)").rearrange(
                "(p q) -> p q", q=OD * F)
            dov = deg4[0:1, :].rearrange("o n -> (o n)").rearrange(
                "(p q) -> p q", q=OD)
            sov = s_own_dram[:].rearrange("n f -> (n f)").rearrange(
                "(p q) -> p q", q=OD * F)
            xo = spool.tile([128, OD, F], F32, tag="xo")
            nc.sync.dma_start(out=xo[:].rearrange("p a b -> p (a b)"), in_=xov[:])
            doi = spool.tile([128, OD], I32, tag="doi")
            nc.sync.dma_start(out=doi[:], in_=dov[:])
            dof = spool.tile([128, OD], F32, tag="dof")
            nc.vector.tensor_copy(out=dof[:], in_=doi[:])
            dor = spool.tile([128, OD], F32, tag="dor")
            nc.vector.reciprocal(out=dor[:], in_=dof[:])
            dos = spool.tile([128, OD], F32, tag="dos")
            nc.scalar.activation(dos[:], dor[:],
                                 mybir.ActivationFunctionType.Sqrt)
            so = spool.tile([128, OD, F], BF16, tag="so")
            nc.vector.tensor_mul(
                out=so[:], in0=xo[:],
                in1=dos[:].unsqueeze(2).to_broadcast([128, OD, F]))
            nc.sync.dma_start(out=sov[:], in_=so[:].rearrange("p a b -> p (a b)"))

            # ---- phase 2: aggregation + epilogue per 4-window group ----
            pool_acc = pacc.tile([1, 128], F32)
            n_pool_mm = NG * WG
            mm_i = 0
            for g in range(NG):
                gi = mpool.tile([128, B], I32, tag="gi")
                nc.sync.dma_start(out=gi[:], in_=gidx_t[g])
                lc = mpool.tile([128, B], F32, tag="lc")
                nc.sync.dma_start(out=lc[:], in_=lcol_t[g])
                lb = mpool.tile([128, WG], F32, tag="lb")
                nc.sync.dma_start(out=lb[:], in_=lbat_t[g])
                d4i = mpool.tile([F, WG * W], I32, tag="d4i")
                nc.sync.dma_start(out=d4i[:],
                                  in_=deg4[:, g * WG * W:(g + 1) * WG * W])

                # gather messages: m[p, b, :] = s[gidx[p, b]]
                # (HW indirect DMA supports one offset per partition, so one
                #  instruction per 128-edge chunk)
                m = mpool.tile([128, B, F], BF16, tag="m")
                for b in range(B):
                    nc.gpsimd.indirect_dma_start(
                        out=m[:, b, :],
                        out_offset=None,
                        in_=s_dram[:],
                        in_offset=IndirectOffsetOnAxis(ap=gi[:, b:b + 1], axis=0),
                    )

                # one-hot matrices
                oh = mpool.tile([128, B, W], BF16, tag="oh")
                nc.vector.tensor_tensor(
                    out=oh[:],
                    in0=lc[:].unsqueeze(2).to_broadcast([128, B, W]),
                    in1=iota_sb[:].unsqueeze(1).to_broadcast([128, B, W]),
                    op=mybir.AluOpType.is_equal)
                bh = mpool.tile([128, WG, 128], BF16, tag="bh")
                nc.vector.tensor_tensor(
                    out=bh[:],
                    in0=lb[:].unsqueeze(2).to_broadcast([128, WG, 128]),
                    in1=iota_sb[:].unsqueeze(1).to_broadcast([128, WG, 128]),
                    op=mybir.AluOpType.is_equal)

                # dis for own nodes of this group
                d4f = mpool.tile([F, WG * W], F32, tag="d4f")
                nc.vector.tensor_copy(out=d4f[:], in_=d4i[:])
                d4r = mpool.tile([F, WG * W], F32, tag="d4r")
                nc.vector.reciprocal(out=d4r[:], in_=d4f[:])
                d4s = mpool.tile([F, WG * W], F32, tag="d4s")
                nc.scalar.activation(d4s[:], d4r[:],
                                     mybir.ActivationFunctionType.Sqrt)

                # self-loop messages for this group's 4 windows: [128, WG*F]
                sm = mpool.tile([128, WG, F], BF16, tag="sm")
                nc.sync.dma_start(
                    out=sm[:],
                    in_=s_own_dram[:].rearrange("n f -> (n f)").rearrange(
                        "(g w p f) -> g w p f", w=WG, p=W, f=F)[g]
                        .transpose([1, 0, 2]))

                # aggregate: z[f, node] += m_chunk^T @ onehot  (+ self term)
                zp = pw.tile([F, WG * W], F32, tag="zp")
                for b in range(B):
                    wb = b // C
                    nc.tensor.matmul(
                        out=zp[:, wb * W:(wb + 1) * W],
                        lhsT=m[:, b, :],
                        rhs=oh[:, b, :],
                        start=(b % C == 0),
                        stop=False)
                    if b % C == C - 1:
                        nc.tensor.matmul(
                            out=zp[:, wb * W:(wb + 1) * W],
                            lhsT=sm[:, wb, :],
                            rhs=ident_sb[:],
                            start=False,
                            stop=True)

                if dbg and g == 0:
                    for nm, t in (("dbg_m", m), ("dbg_oh", oh)):
                        tmp = mpool.tile([128, t[:].free_size()], F32, tag="dbgtmp" + nm)
                        nc.vector.tensor_copy(out=tmp[:], in_=t[:].rearrange("p a b -> p (a b)"))
                        nc.sync.dma_start(out={"dbg_m": dbg_m, "dbg_oh": dbg_oh}[nm][:], in_=tmp[:])
                    tmpb = mpool.tile([128, WG * 128], F32, tag="dbgtmpb")
                    nc.vector.tensor_copy(out=tmpb[:], in_=bh[:].rearrange("p a b -> p (a b)"))
                    nc.sync.dma_start(out=dbg_bh[:], in_=tmpb[:])
                    tmpz = mpool.tile([F, WG * W], F32, tag="dbgtmpz")
                    nc.vector.tensor_copy(out=tmpz[:], in_=zp[:])
                    nc.sync.dma_start(out=dbg_zp[:], in_=tmpz[:])

                # z * dis  (bf16)
                zd = mpool.tile([F, WG * W], BF16, tag="zd")
                nc.vector.tensor_mul(out=zd[:], in0=zp[:], in1=d4s[:])

                # conv = W1^T @ zd : [64, WG*W]
                cv = pw.tile([H, WG * W], F32, tag="cv")
                for w in range(WG):
                    nc.tensor.matmul(
                        out=cv[:, w * W:(w + 1) * W],
                        lhsT=w1b[:],
                        rhs=zd[:, w * W:(w + 1) * W],
                        start=True, stop=True)

                # elu(cv + b1) = relu(x) - relu(1 - exp(x))
                ex = mpool.tile([H, WG * W], F32, tag="ex")
                nc.scalar.activation(ex[:], cv[:],
                                     mybir.ActivationFunctionType.Exp,
                                     bias=b1dup[:])
                r1 = mpool.tile([H, WG * W], F32, tag="r1")
                nc.scalar.activation(r1[:], cv[:],
                                     mybir.ActivationFunctionType.Relu,
                                     bias=b1dup[:])
                r2 = mpool.tile([H, WG * W], F32, tag="r2")
                nc.scalar.activation(r2[:], ex[:],
                                     mybir.ActivationFunctionType.Relu,
                                     scale=-1.0, bias=1.0)
                el = mpool.tile([H, WG * W], BF16, tag="el")
                nc.vector.tensor_sub(out=el[:], in0=r1[:], in1=r2[:])

                if dbg and g == 0:
                    tmz = mpool.tile([F, WG * W], F32, tag="dbgtmz")
                    nc.vector.tensor_copy(out=tmz[:], in_=zd[:])
                    nc.sync.dma_start(out=dbg_zd[:], in_=tmz[:])
                    tme = mpool.tile([H, WG * W], F32, tag="dbgtme")
                    nc.vector.tensor_copy(out=tme[:], in_=el[:])
                    nc.sync.dma_start(out=dbg_el[:], in_=tme[:])

                # q[node] = elu^T @ W2 : [128, WG]
                qp = pw.tile([128, WG], F32, tag="qp")
                for w in range(WG):
                    nc.tensor.matmul(
                        out=qp[:, w:w + 1],
                        lhsT=el[:, w * W:(w + 1) * W],
                        rhs=w2b[:],
                        start=True, stop=True)
                qs = mpool.tile([128, WG], BF16, tag="qs")
                nc.vector.tensor_copy(out=qs[:], in_=qp[:])
                if dbg and g == 0:
                    tmq = mpool.tile([128, WG], F32, tag="dbgtmq")
                    nc.vector.tensor_copy(out=tmq[:], in_=qp[:])
                    nc.sync.dma_start(out=dbg_qs[:], in_=tmq[:])

                # pooled[g'] += q^T @ bhot ; accumulate across all windows
                for w in range(WG):
                    nc.tensor.matmul(
                        out=pool_acc[:],
                        lhsT=qs[:, w:w + 1],
                        rhs=bh[:, w, :],
                        start=(mm_i == 0),
                        stop=(mm_i == n_pool_mm - 1))
                    mm_i += 1

            # ---- finalize: + b2, write out ----
            ob = mpool.tile([1, 128], F32, tag="ob")
            nc.vector.tensor_tensor(
                out=ob[:],
                in0=pool_acc[:],
                in1=b2sb[:].to_broadcast([1, 128]),
                op=mybir.AluOpType.add)
            nc.sync.dma_start(out=outp[:], in_=ob[:])

    return nc


# --------------------------------------------------------------------------
# Entry point
# --------------------------------------------------------------------------

def kernel(x, W1, b1, W2, b2, edge_index, batch):
    # hardcoded problem: N=1M, E=4M, G=1024
    x = np.asarray(x, dtype=np.float32)
    W1 = np.asarray(W1, dtype=np.float32)
    b1 = np.asarray(b1, dtype=np.float32)
    W2 = np.asarray(W2, dtype=np.float32)
    b2 = np.asarray(b2, dtype=np.float32)
    edge_index = np.asarray(edge_index)
    batch = np.asarray(batch)
    n_graphs = 1024

    cfg, percore, shared = host_prep(x, edge_index, batch, n_graphs)

    nc = bacc.Bacc()
    build_kernel(nc, cfg)
    nc.compile()

    in_maps = []
    for c in range(NCORES):
        in_maps.append({
            "x_full": shared["x_pad"],
            "deg_full": shared["deg_full"],
            "deg4": percore["deg4"][c],
            "gidx_t": percore["gidx_t"][c],
            "lcol_t": percore["lcol_t"][c],
            "lbat_t": percore["lbat_t"][c],
            "iota": shared["iota"],
            "ident": shared["ident"],
            "x_own": percore["x_own"][c],
            "W1": W1,
            "b1": b1.reshape(H, 1),
            "W2": W2,
            "b2": b2.reshape(1, 1),
        })

    from concourse.bass_utils import run_bass_kernel_spmd
    trace = bool(int(os.environ.get("KERNEL_TRACE", "0")))
    kw = {}
    if trace:
        kw = dict(trace=True, tmpdir=os.environ.get("KERNEL_TRACE_DIR") or None)
    res = run_bass_kernel_spmd(nc, in_maps, list(range(NCORES)), **kw)
    global LAST_RESULTS
    LAST_RESULTS = res
    gpc = cfg["gpc"]
    out = np.concatenate([res.results[c]["outp"][0, :gpc] for c in range(NCORES)])
    return out.reshape(-1, 1).astype(np.float32)


if __name__ == "__main__":
    pass

